# revision 16
# baseline (speedup 1.0000x reference)
"""OODGAT 2-layer GNN kernel for 8 Trainium2 NeuronCores.

Strategy (dst-partitioned graph parallelism, v3):
  - Nodes sharded across 8 cores (6250 each); edges (incl. folded self-loops)
    partitioned by dst core, laid out in 128-lane x T-tile slot grids sorted
    by destination block.
  - Per layer: node-parallel matmul produces a bf16 row table [sh | msg];
    an AllGather replicates it; per-edge rows are fetched by indirect DMA
    gathers (128 rows per instruction).
  - Segment softmax in unnormalized num/den form (exp max-shift cancels:
    sh = tanh(z/2), logits = 0.5*sh_i*sh_j + 0.5, constants cancel), so
    acc = [den | num] accumulates via one-hot scatter matmuls per 128-dst
    block; the dst-side sh is expanded to edge lanes with per-tile
    transposed-one-hot matmuls; w = exp(0.5*shs*shd) is folded into the
    gathered tile in place.
"""
import numpy as np
import ml_dtypes
from dataclasses import dataclass

import concourse.bass as bass
import concourse.bacc as bacc
import concourse.mybir as mybir
import concourse.tile as tile
from concourse.bass_utils import run_bass_kernel_spmd

F32 = mybir.dt.float32
BF16 = mybir.dt.bfloat16
I32 = mybir.dt.int32
BF = ml_dtypes.bfloat16


@dataclass
class Cfg:
    N: int = 50000
    IN: int = 256
    H: int = 4
    C1: int = 32
    C2: int = 8
    NC: int = 8
    CH_MAX: int = 64   # max tiles per chunk

    @property
    def D1(self):
        return self.H * self.C1

    @property
    def D2(self):
        return self.H * self.C2

    @property
    def SH(self):
        assert self.N % self.NC == 0
        return self.N // self.NC

    @property
    def NBLK(self):
        return (self.SH + 127) // 128


def host_prep(cfg: Cfg, x, edge_index, W1, att1, b1, W2, att2, b2):
    """Shard inputs; build per-core slot layouts. Returns (in_maps, layout)."""
    N, SH, NBLK, NC = cfg.N, cfg.SH, cfg.NBLK, cfg.NC
    src = np.asarray(edge_index[0], dtype=np.int64)
    dst = np.asarray(edge_index[1], dtype=np.int64)
    # self-loops handled by a per-block identity tile (plain DMA, no gather)

    core_of = dst // SH
    counts = np.zeros((NC, NBLK), np.int64)
    per_core = []
    for c in range(NC):
        m = core_of == c
        s_c, d_c = src[m], dst[m]
        ld = d_c - c * SH
        order = np.argsort(ld, kind="stable")
        s_c, ld = s_c[order], ld[order]
        blk = ld // 128
        counts[c] = np.bincount(blk, minlength=NBLK)
        per_core.append((s_c, ld, blk))

    kb = -(-counts.max(axis=0) // 128)
    tb0 = np.concatenate([[0], np.cumsum(kb)])
    T = int(tb0[-1])

    # chunks of whole blocks
    chunks = []   # (b0, b1, t0, nt)
    b0 = 0
    for b in range(NBLK):
        nt = int(tb0[b + 1] - tb0[b0])
        if nt > cfg.CH_MAX and b > b0:
            chunks.append((b0, b, int(tb0[b0]), int(tb0[b] - tb0[b0])))
            b0 = b
    chunks.append((b0, NBLK, int(tb0[b0]), int(tb0[NBLK] - tb0[b0])))

    iota_row = np.broadcast_to(np.arange(128, dtype=np.float32),
                               (128, 128)).astype(BF)
    ident_b = np.eye(128, dtype=np.float32).astype(BF)
    att1_b = np.broadcast_to(np.asarray(att1, np.float32).reshape(1, cfg.D1),
                             (128, cfg.D1)).copy()
    att2_b = np.broadcast_to(np.asarray(att2, np.float32).reshape(1, cfg.D2),
                             (128, cfg.D2)).copy()
    b1_b = np.broadcast_to(np.asarray(b1, np.float32).reshape(1, cfg.D1),
                           (128, cfg.D1)).copy()
    b2_b = np.broadcast_to(np.asarray(b2, np.float32).reshape(1, cfg.C2),
                           (128, cfg.C2)).copy()
    W1_b = np.asarray(W1, np.float32).astype(BF)
    W2_b = np.asarray(W2, np.float32).astype(BF)
    x = np.asarray(x, np.float32)

    in_maps = []
    for c in range(NC):
        s_c, ld, blk = per_core[c]
        goff = np.zeros((128, T), np.int32)
        dstl = np.full((128, T), -1.0, np.float32)
        start = 0
        for b in range(NBLK):
            n = int(counts[c, b])
            e_s = s_c[start:start + n]
            e_l = ld[start:start + n] - b * 128
            start += n
            t0 = int(tb0[b])
            lanes = np.arange(n) % 128
            cols = t0 + np.arange(n) // 128
            goff[lanes, cols] = e_s
            dstl[lanes, cols] = e_l
        xT = np.ascontiguousarray(x[c * SH:(c + 1) * SH].T).astype(BF)
        in_maps.append(dict(
            xT=xT, goff=goff, dstl=dstl.astype(BF),
            iota=iota_row, ident=ident_b, W1=W1_b, W2=W2_b,
            att1_b=att1_b, att2_b=att2_b, b1_b=b1_b, b2_b=b2_b,
        ))
    layout = (tuple(int(v) for v in kb), tuple(chunks))
    return in_maps, layout


def build(cfg: Cfg, layout):
    kb, chunks = layout
    N, SH, NBLK, NC = cfg.N, cfg.SH, cfg.NBLK, cfg.NC
    IN, H, C1, C2, D1, D2 = cfg.IN, cfg.H, cfg.C1, cfg.C2, cfg.D1, cfg.D2
    RW1 = H + D1            # 132: [den/sh 4 | num/msg 128]
    RW2 = H + D2            # 36
    T = sum(kb)
    tb0 = [0]
    for v in kb:
        tb0.append(tb0[-1] + v)
    CHMAX = max(c[3] for c in chunks)
    KBMAX = max(kb)
    KIN = IN // 128

    nc = bacc.Bacc("TRN2", target_bir_lowering=False, debug=False,
                   enable_asserts=True, num_devices=NC)

    xT_in = nc.dram_tensor("xT", [IN, SH], BF16, kind="ExternalInput")
    goff_in = nc.dram_tensor("goff", [128, T], I32, kind="ExternalInput")
    dstl_in = nc.dram_tensor("dstl", [128, T], BF16, kind="ExternalInput")
    iota_in = nc.dram_tensor("iota", [128, 128], BF16, kind="ExternalInput")
    ident_in = nc.dram_tensor("ident", [128, 128], BF16, kind="ExternalInput")
    W1_in = nc.dram_tensor("W1", [IN, D1], BF16, kind="ExternalInput")
    W2_in = nc.dram_tensor("W2", [D1, D2], BF16, kind="ExternalInput")
    att1_in = nc.dram_tensor("att1_b", [128, D1], F32, kind="ExternalInput")
    att2_in = nc.dram_tensor("att2_b", [128, D2], F32, kind="ExternalInput")
    b1_in = nc.dram_tensor("b1_b", [128, D1], F32, kind="ExternalInput")
    b2_in = nc.dram_tensor("b2_b", [128, C2], F32, kind="ExternalInput")
    out = nc.dram_tensor("out", [SH, C2], F32, kind="ExternalOutput")

    T1S = nc.dram_tensor("T1S", [SH, RW1], BF16, kind="Internal")
    T1F = nc.dram_tensor("T1F", [N, RW1], BF16, kind="Internal",
                         addr_space="Shared")
    T2S = nc.dram_tensor("T2S", [SH, RW2], BF16, kind="Internal")
    T2F = nc.dram_tensor("T2F", [N, RW2], BF16, kind="Internal",
                         addr_space="Shared")

    with tile.TileContext(nc) as tc:
        with tc.tile_pool(name="res", bufs=1) as res, \
             tc.tile_pool(name="gp", bufs=4) as gp, \
             tc.tile_pool(name="sp", bufs=3) as sp, \
             tc.tile_pool(name="stp", bufs=6) as stp, \
             tc.tile_pool(name="wk", bufs=3) as wk, \
             tc.tile_pool(name="sm", bufs=4) as sm, \
             tc.tile_pool(name="ps_acc", bufs=2, space="PSUM") as ps_acc, \
             tc.tile_pool(name="ps_sd", bufs=2, space="PSUM") as ps_sd, \
             tc.tile_pool(name="ps_mm", bufs=2, space="PSUM") as ps_mm, \
             tc.tile_pool(name="ps_tp", bufs=2, space="PSUM") as ps_tp:

            # ---- resident constants
            goff = res.tile([128, T], I32)
            dstl = res.tile([128, T], BF16)
            iota = res.tile([128, 128], BF16)
            identb = res.tile([128, 128], BF16)
            att1b = res.tile([128, D1], F32)
            att2b = res.tile([128, D2], F32)
            b1b = res.tile([128, D1], F32)
            b2b = res.tile([128, C2], F32)
            W2sb = res.tile([D1, D2], BF16)
            nc.sync.dma_start(goff[:], goff_in[:])
            nc.sync.dma_start(dstl[:], dstl_in[:])
            nc.sync.dma_start(iota[:], iota_in[:])
            nc.sync.dma_start(identb[:], ident_in[:])
            nc.sync.dma_start(att1b[:], att1_in[:])
            nc.sync.dma_start(att2b[:], att2_in[:])
            nc.sync.dma_start(b1b[:], b1_in[:])
            nc.sync.dma_start(b2b[:], b2_in[:])
            nc.sync.dma_start(W2sb[:], W2_in[:])
            xTs, W1s = [], []
            for k in range(KIN):
                t_ = res.tile([128, SH], BF16, tag=f"xT{k}")
                nc.sync.dma_start(t_[:], xT_in[k * 128:(k + 1) * 128, :])
                xTs.append(t_)
                w_ = res.tile([128, D1], BF16, tag=f"W1{k}")
                nc.sync.dma_start(w_[:], W1_in[k * 128:(k + 1) * 128, :])
                W1s.append(w_)

            # ---- phase 1: rows [tanh(.5*h1.att1) | h1] in bf16
            for i in range(NBLK):
                n0 = i * 128
                P = min(128, SH - n0)
                h1ps = ps_mm.tile([128, D1], F32, space="PSUM", tag="mm")
                for k in range(KIN):
                    nc.tensor.matmul(out=h1ps[:P, :], lhsT=xTs[k][:, n0:n0 + P],
                                     rhs=W1s[k][:], start=(k == 0),
                                     stop=(k == KIN - 1))
                tmp = wk.tile([128, D1], F32, tag="tmp")
                nc.vector.tensor_tensor(out=tmp[:P, :], in0=h1ps[:P, :],
                                        in1=att1b[:P, :],
                                        op=mybir.AluOpType.mult)
                s1 = sm.tile([128, H], F32, tag="s1")
                nc.vector.tensor_reduce(
                    out=s1[:P, :],
                    in_=tmp[:P, :].rearrange("p (h c) -> p h c", h=H),
                    axis=mybir.AxisListType.X, op=mybir.AluOpType.add)
                row = wk.tile([128, RW1], BF16, tag="row")
                nc.scalar.activation(out=row[:P, 0:H], in_=s1[:P, :],
                                     func=mybir.ActivationFunctionType.Tanh,
                                     scale=0.5)
                nc.vector.tensor_copy(out=row[:P, H:RW1], in_=h1ps[:P, :])
                nc.sync.dma_start(T1S[n0:n0 + P, :], row[:P, :])

            def edge_layer(full_tbl, shard_tbl, D, C, RW, layer):
                for (b0c, b1c, tc0, nt) in chunks:
                    G = gp.tile([128, CHMAX, RW], BF16, tag="G")
                    for j in range(nt):
                        nc.gpsimd.indirect_dma_start(
                            out=G[:, j, :], out_offset=None, in_=full_tbl[:],
                            in_offset=bass.IndirectOffsetOnAxis(
                                ap=goff[:, tc0 + j:tc0 + j + 1], axis=0))
                    S = sp.tile([128, CHMAX, 128], BF16, tag="S")
                    nc.vector.tensor_tensor(
                        out=S[:, 0:nt, :],
                        in0=dstl[:, tc0:tc0 + nt, None].to_broadcast(
                            [128, nt, 128]),
                        in1=iota[:, None, :].to_broadcast([128, nt, 128]),
                        op=mybir.AluOpType.is_equal)

                    for b in range(b0c, b1c):
                        c0 = tb0[b] - tc0
                        nb = kb[b]
                        nb0 = b * 128
                        P = min(128, SH - nb0)
                        # dst-side sh + full self rows for this block
                        sblk = sm.tile([128, H], BF16, tag="sblk")
                        nc.vector.memset(sblk[:], 0.0)
                        nc.scalar.dma_start(sblk[:P, :],
                                            shard_tbl[nb0:nb0 + P, 0:H])
                        Gs = wk.tile([128, RW1], BF16, tag="Gs")
                        nc.vector.memset(Gs[:], 0.0)
                        nc.scalar.dma_start(Gs[:P, 0:RW],
                                            shard_tbl[nb0:nb0 + P, 0:RW])
                        # self-loop: w = exp(0.5*sh^2), rhss = [w | msg*w]
                        ps_self = sm.tile([128, H], BF16, tag="ps_self")
                        nc.vector.tensor_tensor(out=ps_self[:], in0=sblk[:],
                                                in1=sblk[:],
                                                op=mybir.AluOpType.mult)
                        rhss = wk.tile([128, RW1], BF16, tag="rhss")
                        nc.scalar.activation(
                            out=rhss[:, 0:H], in_=ps_self[:],
                            func=mybir.ActivationFunctionType.Exp, scale=0.5)
                        nc.vector.tensor_tensor(
                            out=rhss[:, H:RW].rearrange(
                                "p (h c) -> p h c", h=H),
                            in0=Gs[:, H:RW].rearrange("p (h c) -> p h c", h=H),
                            in1=rhss[:, 0:H, None].to_broadcast([128, H, C]),
                            op=mybir.AluOpType.mult)
                        # expand to edge lanes: per tile transposed one-hot
                        sdps = ps_sd.tile([128, KBMAX * H], F32, space="PSUM",
                                          tag="sd")
                        for j in range(nb):
                            tp = ps_tp.tile([128, 128], BF16, space="PSUM",
                                            tag="tp")
                            nc.tensor.transpose(out=tp[:], in_=S[:, c0 + j, :],
                                                identity=identb[:])
                            STt = stp.tile([128, 128], BF16, tag="ST")
                            nc.vector.tensor_copy(out=STt[:], in_=tp[:])
                            nc.tensor.matmul(out=sdps[:, j * H:(j + 1) * H],
                                             lhsT=STt[:], rhs=sblk[:],
                                             start=True, stop=True)
                        # w = exp(0.5*shs*shd) in place; then msg *= w
                        nc.vector.tensor_tensor(
                            out=G[:, c0:c0 + nb, 0:H],
                            in0=G[:, c0:c0 + nb, 0:H],
                            in1=sdps[:, 0:nb * H].rearrange(
                                "p (t h) -> p t h", h=H),
                            op=mybir.AluOpType.mult)
                        nc.scalar.activation(
                            out=G[:, c0:c0 + nb, 0:H],
                            in_=G[:, c0:c0 + nb, 0:H],
                            func=mybir.ActivationFunctionType.Exp, scale=0.5)
                        nc.vector.tensor_tensor(
                            out=G[:, c0:c0 + nb, H:RW].rearrange(
                                "p t (h c) -> p t h c", h=H),
                            in0=G[:, c0:c0 + nb, H:RW].rearrange(
                                "p t (h c) -> p t h c", h=H),
                            in1=G[:, c0:c0 + nb, 0:H, None].to_broadcast(
                                [128, nb, H, C]),
                            op=mybir.AluOpType.mult)
                        acc = ps_acc.tile([128, RW1], F32, space="PSUM",
                                          tag="acc")
                        nc.tensor.matmul(out=acc[:, 0:RW], lhsT=identb[:],
                                         rhs=rhss[:, 0:RW], start=True,
                                         stop=(nb == 0))
                        for j in range(nb):
                            nc.tensor.matmul(
                                out=acc[:, 0:RW], lhsT=S[:, c0 + j, :],
                                rhs=G[:, c0 + j, :], start=False,
                                stop=(j == nb - 1))
                        if layer == 1:
                            epilogue1(b, acc)
                        else:
                            epilogue2(b, acc)

            def epilogue1(b, acc):
                nb0 = b * 128
                P = min(128, SH - nb0)
                rz = sm.tile([128, H], F32, tag="rz")
                nc.vector.reciprocal(out=rz[:], in_=acc[:, 0:H])
                o1 = wk.tile([128, D1], F32, tag="o1")
                nc.vector.tensor_tensor(
                    out=o1[:].rearrange("p (h c) -> p h c", h=H),
                    in0=acc[:, H:H + D1].rearrange("p (h c) -> p h c", h=H),
                    in1=rz[:, :, None].to_broadcast([128, H, C1]),
                    op=mybir.AluOpType.mult)
                nc.vector.tensor_tensor(out=o1[:], in0=o1[:], in1=b1b[:],
                                        op=mybir.AluOpType.add)
                # ELU = exp(min(x,0)) - 1 + max(x,0) via ACT relu/exp chain
                r1 = wk.tile([128, D1], F32, tag="r1")
                nc.scalar.activation(out=r1[:], in_=o1[:],
                                     func=mybir.ActivationFunctionType.Relu,
                                     scale=-1.0)
                ew = wk.tile([128, D1], F32, tag="ew")
                nc.scalar.activation(out=ew[:], in_=r1[:],
                                     func=mybir.ActivationFunctionType.Exp,
                                     scale=-1.0)
                rp = wk.tile([128, D1], F32, tag="rp")
                nc.scalar.activation(out=rp[:], in_=o1[:],
                                     func=mybir.ActivationFunctionType.Relu)
                hact = wk.tile([128, D1], BF16, tag="hact")
                nc.vector.scalar_tensor_tensor(
                    out=hact[:], in0=ew[:], scalar=-1.0, in1=rp[:],
                    op0=mybir.AluOpType.add, op1=mybir.AluOpType.add)
                tp = ps_tp.tile([128, 128], BF16, space="PSUM", tag="tp")
                nc.tensor.transpose(out=tp[:], in_=hact[:], identity=identb[:])
                hT = stp.tile([128, 128], BF16, tag="ST")
                nc.vector.tensor_copy(out=hT[:], in_=tp[:])
                h2ps = ps_mm.tile([128, D2], F32, space="PSUM", tag="mm")
                nc.tensor.matmul(out=h2ps[:], lhsT=hT[:], rhs=W2sb[:],
                                 start=True, stop=True)
                t2 = sm.tile([128, D2], F32, tag="t2")
                nc.vector.tensor_tensor(out=t2[:], in0=h2ps[:], in1=att2b[:],
                                        op=mybir.AluOpType.mult)
                s2 = sm.tile([128, H], F32, tag="s2")
                nc.vector.tensor_reduce(
                    out=s2[:], in_=t2[:].rearrange("p (h c) -> p h c", h=H),
                    axis=mybir.AxisListType.X, op=mybir.AluOpType.add)
                row2 = wk.tile([128, RW2], BF16, tag="row2")
                nc.scalar.activation(out=row2[:, 0:H], in_=s2[:],
                                     func=mybir.ActivationFunctionType.Tanh,
                                     scale=0.5)
                nc.vector.tensor_copy(out=row2[:, H:RW2], in_=h2ps[:])
                nc.sync.dma_start(T2S[nb0:nb0 + P, :], row2[:P, :])

            def epilogue2(b, acc):
                nb0 = b * 128
                P = min(128, SH - nb0)
                rz = sm.tile([128, H], F32, tag="rz")
                nc.vector.reciprocal(out=rz[:], in_=acc[:, 0:H])
                o2 = sm.tile([128, D2], F32, tag="o2")
                nc.vector.tensor_tensor(
                    out=o2[:].rearrange("p (h c) -> p h c", h=H),
                    in0=acc[:, H:H + D2].rearrange("p (h c) -> p h c", h=H),
                    in1=rz[:, :, None].to_broadcast([128, H, C2]),
                    op=mybir.AluOpType.mult)
                red = sm.tile([128, C2], F32, tag="red")
                nc.vector.tensor_reduce(
                    out=red[:], in_=o2[:].rearrange("p (h c) -> p c h", h=H),
                    axis=mybir.AxisListType.X, op=mybir.AluOpType.add)
                fin = sm.tile([128, C2], F32, tag="fin")
                nc.vector.scalar_tensor_tensor(
                    out=fin[:], in0=red[:], scalar=1.0 / H, in1=b2b[:],
                    op0=mybir.AluOpType.mult, op1=mybir.AluOpType.add)
                nc.sync.dma_start(out[nb0:nb0 + P, :], fin[:P, :])

            # ---- layer 1
            nc.gpsimd.collective_compute(
                "AllGather", mybir.AluOpType.bypass,
                replica_groups=[list(range(NC))],
                ins=[T1S[:]], outs=[T1F[:]])
            edge_layer(T1F, T1S, D1, C1, RW1, 1)

            # ---- layer 2
            nc.gpsimd.collective_compute(
                "AllGather", mybir.AluOpType.bypass,
                replica_groups=[list(range(NC))],
                ins=[T2S[:]], outs=[T2F[:]])
            edge_layer(T2F, T2S, D2, C2, RW2, 2)

    nc.compile()
    return nc


_CACHE = {}


def kernel(x, edge_index, W1, att1, b1, W2, att2, b2, cfg: Cfg | None = None,
           trace: bool = False):
    cfg = cfg or Cfg()
    in_maps, layout = host_prep(cfg, x, edge_index, W1, att1, b1, W2, att2, b2)
    key = (cfg.N, cfg.IN, cfg.H, cfg.C1, cfg.C2, layout[0])
    if key not in _CACHE:
        _CACHE[key] = build(cfg, layout)
    nc = _CACHE[key]
    r = run_bass_kernel_spmd(nc, in_maps, core_ids=list(range(cfg.NC)),
                             trace=trace)
    outp = np.concatenate([r.results[c]["out"] for c in range(cfg.NC)], axis=0)
    if trace:
        kernel.last_exec_time_ns = r.exec_time_ns
    return outp.astype(np.float32)
